# revision 1
# baseline (speedup 1.0000x reference)
"""ClusterNorm2d Trainium2 kernel.

Reference semantics (see problem): per-(cluster, channel) statistics over
(batch members of the cluster) x (spatial), blended 0.2/0.8 with running
stats, then per-sample affine normalization.

Sharding: channel-parallel across the 8 NeuronCores (8 channels each).
Cluster statistics for a channel only ever combine values of that same
channel across the batch, so each core computes its channels' statistics
independently -- no cross-core collective is needed at all.

Per-core layout: the [64, 8, 112, 112] channel shard is viewed
channel-major as [512 rows = (c, b), 12544 = H*W] in 4 SBUF-resident
tiles of [128, 12544] f32. Each tile holds 2 *complete* channels
(2 x 64 batch rows), so its cluster statistics are self-contained:
tile t's normalized output DMA can start while tiles t+1.. are still
streaming in, and the DMA engines stay busy back-to-back at the HBM
roofline (x is read from HBM exactly once, y written once).

Per tile:
  DMA in -> DVE row sums + ACT Square accum_out chunks (sum of squares)
  -> tiny PE matmul vs host-built one-hot (segment-sum over batch)
  -> tiny stats chain (blend, sqrt, reciprocal) [all label/count math
     folded on host into per-(channel,cluster) coefficient vectors]
  -> tiny PE matmul gather (per-row scale/offset)
  -> in-place fused DVE affine (x*scale + offset, 2x DVE mode) -> DMA out.
"""

import os
import sys

import numpy as np

for _p in (
    "/opt/trn_rl_repo",
    "/root/.axon_site",
    "/root/.axon_site/_ro/pypackages",
):
    if _p not in sys.path and os.path.isdir(_p):
        sys.path.append(_p)

import concourse.bacc as bacc
import concourse.bass as bass
import concourse.tile as tile
from concourse import mybir
from concourse.bass_utils import run_bass_kernel_spmd

EPS = 1e-05
N_CLUSTERS = 4
B, C, H, W = 64, 64, 112, 112
HW = H * W                      # 12544
N_CORES = 8
CS = C // N_CORES               # 8 channels per core
R = B * CS                      # 512 rows per core
P = 128                         # SBUF partitions
NT = R // P                     # 4 row tiles per core
CT = P // B                     # 2 channels per tile
GC = N_CLUSTERS * CT            # 8 (channel, cluster) pairs per tile
SQ_CHUNK = 896                  # ACT square chunk (fits 2 PSUM banks)
NCH = HW // SQ_CHUNK            # 14 chunks
RED_B = 128                     # inner width of 2-level row-sum reduce
RED_A = HW // RED_B             # 98

_F32 = mybir.dt.float32

_CACHE = {}


def _build_nc(n_iters=1, variant="full"):
    """Build + compile the single-core Bass program (SPMD across 8 cores).

    n_iters > 1 repeats the whole body (used only for benchmarking: the
    in-NEFF loop lets per-iteration HW time be measured as a wall-clock
    delta, cancelling the PJRT/axon dispatch overhead).
    """
    nc = bacc.Bacc("TRN2", target_bir_lowering=False, debug=False)

    x = nc.dram_tensor("x", [R, HW], _F32, kind="ExternalInput")
    oh = nc.dram_tensor("oh", [NT, P, GC], _F32, kind="ExternalInput")
    gs = nc.dram_tensor("gs", [NT, GC, P], _F32, kind="ExternalInput")
    par = nc.dram_tensor("par", [NT * GC, 16], _F32, kind="ExternalInput")
    y = nc.dram_tensor("y", [R, HW], _F32, kind="ExternalOutput")

    with tile.TileContext(nc) as tc:
        with (
            tc.tile_pool(name="consts", bufs=1) as consts,
            tc.tile_pool(
                name="xpool",
                bufs=(NT // 2 if variant in ("pairin", "pairboth") else NT),
            ) as xpool,
            tc.tile_pool(name="stats", bufs=2 * NT) as stats,
            tc.tile_pool(name="pscr", bufs=2, space="PSUM") as pscr,
            tc.tile_pool(name="pacc", bufs=2, space="PSUM") as pacc,
            tc.tile_pool(name="psc", bufs=2, space="PSUM") as psc,
        ):
            sb_oh = consts.tile([P, NT, GC], _F32)
            nc.sync.dma_start(out=sb_oh, in_=oh.rearrange("t k j -> k t j"))
            sb_gs = consts.tile([GC, NT, P], _F32)
            nc.sync.dma_start(out=sb_gs, in_=gs.rearrange("t j k -> j t k"))
            sb_par = consts.tile([GC, NT, 16], _F32)
            nc.sync.dma_start(
                out=sb_par, in_=par.rearrange("(t j) c -> j t c", j=GC)
            )
            pools = (xpool, stats, pscr, pacc, psc)
            for _ in range(n_iters):
                if variant == "memcpy":
                    _emit_memcpy_iter(nc, x, y, xpool)
                elif variant in ("pairin", "pairboth"):
                    _emit_pair_iter(nc, x, y, sb_oh, sb_gs, sb_par, pools,
                                    variant)
                else:
                    _emit_iter(nc, x, y, sb_oh, sb_gs, sb_par, pools, variant)

    nc.compile()
    return nc


def _emit_memcpy_iter(nc, x, y, xpool):
    """DMA in + DMA out only, same trigger order as the full kernel
    (4 loads then 4 stores) — measures the pure memory roofline."""
    xt = []
    for t in range(NT):
        xtile = xpool.tile([P, HW], _F32, tag="x")
        nc.sync.dma_start(out=xtile, in_=x[t * P:(t + 1) * P, :])
        xt.append(xtile)
    for t in range(NT):
        nc.sync.dma_start(out=y[t * P:(t + 1) * P, :], in_=xt[t])


def _emit_iter(nc, x, y, sb_oh, sb_gs, sb_par, pools, variant="full"):
    xpool, stats, pscr, pacc, psc = pools
    AX = mybir.AxisListType.X
    ADD = mybir.AluOpType.add
    MUL = mybir.AluOpType.mult
    HH = HW // 2

    xt = []
    for t in range(NT):
        xtile = xpool.tile([P, HW], _F32, tag="x")
        rows = slice(t * P, (t + 1) * P)
        if variant == "split2":
            nc.sync.dma_start(out=xtile[:, 0:HH], in_=x[rows, 0:HH])
            nc.sync.dma_start(out=xtile[:, HH:HW], in_=x[rows, HH:HW])
        elif variant == "ring2in" and t % 2 == 1:
            # odd tiles loaded via the ACT HWDGE ring (2nd descriptor path)
            nc.scalar.dma_start(out=xtile, in_=x[rows, :])
        elif variant == "swin":
            # loads via the SWDGE (gpsimd/Q7) path; stores stay on HWDGE,
            # so iteration-boundary loads can drain concurrently with
            # still-draining stores on the other queue set
            nc.gpsimd.dma_start(out=xtile, in_=x[rows, :])
        else:
            nc.sync.dma_start(out=xtile, in_=x[rows, :])
        xt.append(xtile)

    for t in range(NT):
        _emit_tile_compute(nc, t, xt[t], sb_oh, sb_gs, sb_par,
                           stats, pscr, pacc, psc,
                           stats_mode=("bnstats" if variant == "bnstats"
                                       else "sums"))
        rows = slice(t * P, (t + 1) * P)
        if variant == "out_act":
            nc.scalar.dma_start(out=y[rows, :], in_=xt[t])
        elif variant == "split2":
            nc.sync.dma_start(out=y[rows, 0:HH], in_=xt[t][:, 0:HH])
            nc.sync.dma_start(out=y[rows, HH:HW], in_=xt[t][:, HH:HW])
        else:
            nc.sync.dma_start(out=y[rows, :], in_=xt[t])


def _emit_pair_iter(nc, x, y, sb_oh, sb_gs, sb_par, pools, variant):
    """Tiles loaded (and optionally stored) in pairs: 2x12.8MB DMA streams
    instead of 4x6.4MB, testing whether longer streams raise HBM efficiency."""
    xpool, stats, pscr, pacc, psc = pools
    NP = NT // 2
    xp = []
    for pt in range(NP):
        xtile = xpool.tile([P, 2, HW], _F32, tag="xpair")
        nc.sync.dma_start(
            out=xtile,
            in_=x[pt * 2 * P:(pt + 1) * 2 * P, :].rearrange(
                "(r p) w -> p r w", p=P
            ),
        )
        xp.append(xtile)

    for pt in range(NP):
        for r in range(2):
            t = pt * 2 + r
            _emit_tile_compute(nc, t, xp[pt][:, r, :], sb_oh, sb_gs, sb_par,
                               stats, pscr, pacc, psc)
            if variant == "pairin":
                rows = slice(t * P, (t + 1) * P)
                nc.sync.dma_start(out=y[rows, :], in_=xp[pt][:, r, :])
        if variant == "pairboth":
            nc.sync.dma_start(
                out=y[pt * 2 * P:(pt + 1) * 2 * P, :].rearrange(
                    "(r p) w -> p r w", p=P
                ),
                in_=xp[pt],
            )


def _emit_tile_compute(nc, t, xv, sb_oh, sb_gs, sb_par, stats, pscr, pacc, psc,
                       stats_mode="sums"):
    """Stats + normalization for one logical 128-row tile; xv is its [P, HW]
    SBUF view, updated in place."""
    AX = mybir.AxisListType.X
    ADD = mybir.AluOpType.add
    MUL = mybir.AluOpType.mult

    if stats_mode == "bnstats":
        # One DVE read of x yields mean AND variance per row (no ACT squares,
        # halving SBUF engine-read pressure while DMA streams tiles in).
        fmax = nc.vector.BN_STATS_FMAX                    # 512
        n_full, rem = divmod(HW, fmax)                    # 24, 256
        n_sub = n_full + (1 if rem else 0)
        bst = stats.tile([P, n_sub, nc.vector.BN_STATS_DIM], _F32, tag="bst")
        for i in range(n_sub):
            c0 = i * fmax
            w = fmax if i < n_full else rem
            nc.vector.bn_stats(out=bst[:, i, :], in_=xv[:, c0:c0 + w])
        mv = stats.tile([P, 2], _F32, tag="s_ss")         # (mean, var)
        nc.vector.bn_aggr(out=mv, in_=bst)
        q2t = stats.tile([P, 1], _F32, tag="q2t")
        nc.vector.tensor_mul(q2t, mv[:, 0:1], mv[:, 0:1])
        nc.vector.tensor_add(mv[:, 1:2], mv[:, 1:2], q2t)  # q = var + m^2
        ss_t = mv                                          # rhs = (m, q)
    else:
        # --- per-row sum and sum-of-squares --------------------------------
        ss_t = stats.tile([P, 2], _F32, tag="s_ss")
        part = stats.tile([P, RED_A], _F32, tag="part")
        nc.vector.tensor_reduce(
            part,
            xv.rearrange("p (a b) -> p a b", b=RED_B),
            axis=AX,
            op=ADD,
        )
        nc.vector.tensor_reduce(ss_t[:, 0:1], part, axis=AX, op=ADD)

        sqp = stats.tile([P, NCH], _F32, tag="sqp")
        for ch in range(NCH):
            scr = pscr.tile([P, SQ_CHUNK], _F32, tag="scr")
            nc.scalar.activation(
                out=scr,
                in_=xv[:, ch * SQ_CHUNK:(ch + 1) * SQ_CHUNK],
                func=mybir.ActivationFunctionType.Square,
                accum_out=sqp[:, ch:ch + 1],
            )
        nc.vector.tensor_reduce(ss_t[:, 1:2], sqp, axis=AX, op=ADD)

    # --- segment-sum over the 64 batch rows of each channel ----------------
    psum_acc = pacc.tile([GC, 2], _F32, tag="acc")
    nc.tensor.matmul(
        psum_acc, lhsT=sb_oh[:, t, :], rhs=ss_t, start=True, stop=True
    )

    # --- cluster stats -> per-(channel,cluster) scale/offset ---------------
    # par columns (sums mode): 0:c_mean 1:cA 2:cB 3:rv08(+eps) 4:rm08 5:w 6:b
    # bnstats mode uses 8:c_mean_bn 9:cA_bn instead of 0/1 (cB shared).
    pt = sb_par[:, t, :]
    if stats_mode == "bnstats":
        pt_cmean, pt_cA = pt[:, 8:9], pt[:, 9:10]
    else:
        pt_cmean, pt_cA = pt[:, 0:1], pt[:, 1:2]
    st = stats.tile([GC, 8], _F32, tag="st")
    so8 = stats.tile([GC, 2], _F32, tag="so8")
    mean = st[:, 0:1]
    q2 = st[:, 1:2]
    varb = st[:, 2:3]
    tmp = st[:, 3:4]
    std = st[:, 4:5]
    rstd = st[:, 5:6]
    mu = st[:, 6:7]
    nc.vector.tensor_mul(mean, psum_acc[:, 0:1], pt_cmean)
    nc.vector.tensor_mul(q2, mean, mean)
    nc.vector.tensor_mul(varb, psum_acc[:, 1:2], pt_cA)
    nc.vector.tensor_mul(tmp, q2, pt[:, 2:3])
    nc.vector.tensor_sub(varb, varb, tmp)
    nc.vector.tensor_add(varb, varb, pt[:, 3:4])
    nc.scalar.activation(
        out=std, in_=varb, func=mybir.ActivationFunctionType.Sqrt
    )
    nc.vector.reciprocal(rstd, std)
    nc.vector.tensor_mul(so8[:, 0:1], rstd, pt[:, 5:6])
    nc.vector.tensor_scalar(
        out=mu, in0=mean, scalar1=0.2, scalar2=pt[:, 4:5],
        op0=MUL, op1=ADD,
    )
    nc.vector.tensor_mul(tmp, mu, so8[:, 0:1])
    nc.vector.tensor_sub(so8[:, 1:2], pt[:, 6:7], tmp)

    # --- scatter scale/offset to rows, fused in-place affine ---------------
    pso = psc.tile([P, 2], _F32, tag="pso")
    nc.tensor.matmul(
        pso, lhsT=sb_gs[:, t, :], rhs=so8, start=True, stop=True
    )
    so_t = stats.tile([P, 2], _F32, tag="so_t")
    nc.vector.tensor_copy(so_t, pso)
    nc.vector.tensor_scalar(
        out=xv,
        in0=xv,
        scalar1=so_t[:, 0:1],
        scalar2=so_t[:, 1:2],
        op0=MUL,
        op1=ADD,
    )


def host_prep(x, running_mean, running_var, weight, bias, labels):
    """Fold all label math into per-core input tensors. Returns in_maps."""
    labels = np.asarray(labels).astype(np.int64)
    x = np.asarray(x, dtype=np.float32)

    cnt = np.bincount(labels, minlength=N_CLUSTERS).astype(np.float64)
    N = cnt * HW
    c_mean = 1.0 / np.maximum(N, 1.0)
    denom = np.maximum(N - 1.0, 1.0)
    cA = 0.2 / denom
    cB = 0.2 * N / denom
    # bnstats-mode coefficients: device supplies per-row (mean, mean-of-sq)
    # instead of (sum, sum-of-sq), so fold the extra HW factor here.
    c_mean_bn = 1.0 / np.maximum(cnt, 1.0)
    cA_bn = 0.2 * HW / denom

    # Row layout per core: r = cl*B + b (channel-major).  Tile t holds
    # channels {2t, 2t+1}; within the tile, row k -> (cl_local = k//B,
    # b = k%B); stats slot j = cl_local*N_CLUSTERS + g.
    oh = np.zeros((NT, P, GC), dtype=np.float32)
    gs = np.zeros((NT, GC, P), dtype=np.float32)
    k = np.arange(P)
    for t in range(NT):
        j = (k // B) * N_CLUSTERS + labels[k % B]
        oh[t, k, j] = 1.0
        gs[t, j, k] = 1.0

    # par rows: (t, j) -> channel c = core*CS + 2t + j//N_CLUSTERS,
    # cluster g = j % N_CLUSTERS
    jj = np.arange(GC)
    g_of_j = jj % N_CLUSTERS
    rm = np.asarray(running_mean, np.float64)
    rv = np.asarray(running_var, np.float64)
    wt = np.asarray(weight, np.float32)
    bs = np.asarray(bias, np.float32)

    # One big channel-major transpose; per-core shards are then zero-copy
    # contiguous views.
    x_cm = np.ascontiguousarray(x.transpose(1, 0, 2, 3)).reshape(C, B * HW)

    in_maps = []
    for i in range(N_CORES):
        par = np.zeros((NT * GC, 16), dtype=np.float32)
        for t in range(NT):
            c_of_j = i * CS + 2 * t + jj // N_CLUSTERS
            rows = slice(t * GC, (t + 1) * GC)
            par[rows, 0] = c_mean[g_of_j]
            par[rows, 1] = cA[g_of_j]
            par[rows, 2] = cB[g_of_j]
            par[rows, 3] = 0.8 * rv[c_of_j] + EPS
            par[rows, 4] = 0.8 * rm[c_of_j]
            par[rows, 5] = wt[c_of_j]
            par[rows, 6] = bs[c_of_j]
            par[rows, 8] = c_mean_bn[g_of_j]
            par[rows, 9] = cA_bn[g_of_j]
        xs = x_cm[i * CS:(i + 1) * CS].reshape(R, HW)
        in_maps.append({"x": xs, "oh": oh, "gs": gs, "par": par})
    return in_maps


def get_nc(n_iters=1, variant="full"):
    key = ("nc", n_iters, variant)
    if key not in _CACHE:
        _CACHE[key] = _build_nc(n_iters, variant)
    return _CACHE[key]


def assemble_out(per_core_y):
    """[N_CORES] x [R, HW] channel-major shards -> [B, C, H, W] (a view)."""
    full = np.concatenate(
        [yc.reshape(CS, B, H, W) for yc in per_core_y], axis=0
    )  # [C, B, H, W]
    return full.transpose(1, 0, 2, 3)


def kernel(x, running_mean, running_var, weight, bias, labels, **run_kwargs):
    nc = get_nc()
    in_maps = host_prep(x, running_mean, running_var, weight, bias, labels)
    res = run_bass_kernel_spmd(nc, in_maps, list(range(N_CORES)), **run_kwargs)
    out = assemble_out([res.results[i]["y"] for i in range(N_CORES)])
    if run_kwargs:
        kernel.last_results = res
    return out

